# revision 3
# baseline (speedup 1.0000x reference)
"""Trainium2 Bass kernel for masked grouped-bottleneck (moe_routing patch refine).

Full computation:
  x [16,1024,56,56] is split into a 7x7 grid of 8x8 patches; per patch a
  grouped (G=4) bottleneck conv1(1x1)->relu->conv2(3x3, per-patch pad)->relu
  ->conv3(1x1) runs; the result is zeroed for non-selected (b, group, patch)
  combos per `mask`, un-patchified, added to x (residual) and relu'd.

Sharding: data-parallel over batch, 2 images per core across 8 cores.

All tensors ride in bf16 (inputs downcast on the host, output upcast back to
fp32 on the host): the 2e-2 rel-err budget dwarfs bf16's ~2e-3, and halving
the HBM bytes moves the DMA floor from ~143us to ~71us per core while the PE
stays at 1 cycle/row either way (fp32r is also 1 cycle/row at N=448).

Weights are repacked on the host into PE-friendly lhsT layouts (block-diagonal
over group pairs so conv2 runs dense K=128/M=128 matmuls). The routing mask is
applied right after conv2: every conv is patch-local and bias-free, so zeroing
m2 for a (group, patch) is exactly equivalent to zeroing the conv3 output.

Pipeline per (batch, group-pair) macro-iteration, streaming 7 patch rows:
  conv1 (PE, 4 accumulating blockdiag matmuls) -> relu (ACT) into the zero-
  padded m1 interior (borders memset once, first 3 jobs only — they are never
  overwritten) -> conv2 as 9 accumulating taps over shifted padded views
  (walrus requires matmul OUT APs to canonicalize to <=3 dims, so clipped
  PSUM sub-views are not an option) -> fused mask*relu (one DVE
  scalar_tensor_tensor m2 = (p2 max 0) * mask) -> conv3 (PE) -> residual add
  (DVE) -> relu into a 4-slab output tile, blocks 0/1 on ACT and blocks 2/3 on
  Pool (gpsimd), keeping both under the PE roofline -> one store per patch row.

Engine budget per core (28 jobs, cost-model): PE 17x448 rows/job = 89us,
DVE (stt + 4 residual adds, PSUM-bound so no 2x mode) = 83us, ACT = 46us,
Pool = 40us, DMA 25.7MB = 71us.

DMA: x loads (one 3.2MB dma_start per set) and the batched stores ride the SP
(sync) HWDGE ring; loads for a set are issued two sets ahead (xpool bufs=3).
Stores must NOT go on the ACT ring: a store's sem wait would hold the ACT
sequencer and block the next row's relu decodes.
"""
import numpy as np

_CACHE = {}

B, C, H, W = 16, 1024, 56, 56
G, MS, HP = 4, 7, 8
MID = 256
NCORES = 8
BPC = B // NCORES   # batches per core


def _bf16():
    from concourse import mybir
    return mybir.dt.np(mybir.dt.bfloat16)


def _pack_weights(w1, w2, w3):
    w1s = np.zeros((128, 2 * 4 * 128), np.float32)
    for pair in range(2):
        for j in range(4):            # K-tile over the pair's 512 input chans
            gi, kt = j // 2, j % 2
            g = 2 * pair + gi
            Wg = w1[64 * g:64 * g + 64, 128 * kt:128 * kt + 128, 0, 0]
            w1s[:, (pair * 4 + j) * 128 + 64 * gi:(pair * 4 + j) * 128 + 64 * gi + 64] = Wg.T
    w2s = np.zeros((128, 2 * 9 * 128), np.float32)
    for pair in range(2):
        for tap in range(9):
            dy, dx = tap // 3, tap % 3
            for gi in range(2):
                g = 2 * pair + gi
                Wg = w2[64 * g:64 * g + 64, :, dy, dx]
                w2s[64 * gi:64 * gi + 64,
                    (pair * 9 + tap) * 128 + 64 * gi:(pair * 9 + tap) * 128 + 64 * gi + 64] = Wg.T
    w3s = np.zeros((128, 8 * 128), np.float32)
    for pair in range(2):
        for gi in range(2):
            g = 2 * pair + gi
            for mt in range(2):
                Wg = w3[256 * g + 128 * mt:256 * g + 128 * (mt + 1), :, 0, 0]
                blk = (pair * 2 + gi) * 2 + mt
                w3s[64 * gi:64 * gi + 64, blk * 128:(blk + 1) * 128] = Wg.T
    return w1s, w2s, w3s


def _pack_mask(mask_b):
    # mask_b: [BPC, 4, 7, 7] -> [128, BPC*2*49], row r belongs to group 2*pair + r//64
    m = np.zeros((128, BPC * 2 * 49), np.float32)
    mb = (mask_b > 0).astype(np.float32).reshape(BPC, 4, 49)
    for b in range(BPC):
        for pair in range(2):
            seg = slice((b * 2 + pair) * 49, (b * 2 + pair + 1) * 49)
            m[0:64, seg] = mb[b, 2 * pair]
            m[64:128, seg] = mb[b, 2 * pair + 1]
    return m


def _build_program(reps=1, store_engine="sync"):
    import concourse.bacc as bacc
    import concourse.mybir as mybir
    import concourse.tile as tile

    f32 = mybir.dt.float32
    bf16 = mybir.dt.bfloat16
    Relu = mybir.ActivationFunctionType.Relu
    Max = mybir.AluOpType.max
    Mult = mybir.AluOpType.mult

    nc = bacc.Bacc("TRN2", target_bir_lowering=False, debug=False)
    x_d = nc.dram_tensor("x", [BPC, C, H, W], bf16, kind="ExternalInput")
    mk_d = nc.dram_tensor("maskrep", [128, BPC * 2 * 49], f32, kind="ExternalInput")
    w1_d = nc.dram_tensor("w1s", [128, 1024], bf16, kind="ExternalInput")
    w2_d = nc.dram_tensor("w2s", [128, 2304], bf16, kind="ExternalInput")
    w3_d = nc.dram_tensor("w3s", [128, 1024], bf16, kind="ExternalInput")
    out_d = nc.dram_tensor("out", [BPC, C, H, W], bf16, kind="ExternalOutput")

    xap = x_d.ap()
    oap = out_d.ap()

    with tile.TileContext(nc) as tc:
        with (
            tc.tile_pool(name="wpool", bufs=1) as wpool,
            tc.tile_pool(name="xpool", bufs=3) as xpool,
            tc.tile_pool(name="m1pool", bufs=3) as m1pool,
            tc.tile_pool(name="m2pool", bufs=3) as m2pool,
            tc.tile_pool(name="t3pool", bufs=3) as t3pool,
            tc.tile_pool(name="opool", bufs=3) as opool,
            tc.tile_pool(name="ps1", bufs=2, space="PSUM") as ps1,
            tc.tile_pool(name="ps2", bufs=2, space="PSUM") as ps2,
            tc.tile_pool(name="ps3", bufs=4, space="PSUM") as ps3,
        ):
            w1t = wpool.tile([128, 1024], bf16, tag="w1")
            w2t = wpool.tile([128, 2304], bf16, tag="w2")
            w3t = wpool.tile([128, 1024], bf16, tag="w3")
            mkt = wpool.tile([128, BPC * 2 * 49], f32, tag="mk")
            # weights go on the ACT HWDGE ring so they don't queue ahead
            # of the first x-tile loads on the sync ring at startup
            nc.scalar.dma_start(w1t[:], w1_d.ap())
            nc.scalar.dma_start(w2t[:], w2_d.ap())
            nc.scalar.dma_start(w3t[:], w3_d.ap())
            nc.scalar.dma_start(mkt[:], mk_d.ap())

            store_eng = nc.scalar if store_engine == "scalar" else nc.sync

            # (b, pair) macro-sets, 7 patch-row jobs each, pipelined 2 deep:
            # iteration i runs conv1[i], relu1+conv2[i-1], and
            # stt+conv3+residual+store[i-2] so no engine queue ever waits
            # on a same-row cross-engine dependency.
            set_list = [(b, pair)
                        for _ in range(reps)
                        for b in range(BPC)
                        for pair in range(2)]
            xts_for_set = {}

            def load_set(si):
                b, pair = set_list[si]
                xt = xpool.tile([128, 4 * H * W], bf16, tag="xt", name=f"xt{si}")
                nc.sync.dma_start(
                    xt[:].rearrange("c (u s) -> c u s", u=4),
                    xap[b, 512 * pair:512 * pair + 512]
                    .rearrange("(u c) h w -> c u (h w)", u=4))
                xts_for_set[si] = xt

            jobs = [(si, py) for si in range(len(set_list)) for py in range(MS)]
            load_set(0)
            if len(set_list) > 1:
                load_set(1)
            state = {}

            for i in range(len(jobs) + 2):
                # ---- stage A (job i): conv1 ----
                if i < len(jobs):
                    si, py = jobs[i]
                    if py == 0 and si + 2 < len(set_list):
                        load_set(si + 2)
                    b, pair = set_list[si]
                    xt = xts_for_set[si]
                    xviews = [
                        xt[:].rearrange("p (u py y px x) -> p u py px y x",
                                        u=4, py=7, y=8, px=7, x=8)[:, u, py]
                        for u in range(4)
                    ]
                    p1 = ps1.tile([128, 448], f32)
                    for j in range(4):
                        nc.tensor.matmul(
                            p1[:],
                            w1t[:, (pair * 4 + j) * 128:(pair * 4 + j + 1) * 128],
                            xviews[j],
                            start=(j == 0), stop=(j == 3))
                    state[i] = dict(si=si, py=py, b=b, pair=pair,
                                    xviews=xviews, p1=p1)
                # ---- stage B (job i-1): relu into padded m1, conv2 ----
                if 0 <= i - 1 < len(jobs):
                    s = state[i - 1]
                    b, pair, py = s["b"], s["pair"], s["py"]
                    m1 = m1pool.tile([128, 700], bf16)
                    if i - 1 < 3:
                        # zero the patch borders once per physical buffer;
                        # only the 8x8 interior is ever rewritten after this
                        nc.gpsimd.memset(m1[:].bitcast(f32), 0.0)
                    m1v = m1[:].rearrange("p (a b c) -> p a b c", b=10, c=10)
                    p1v = s["p1"][:].rearrange("p (a b c) -> p a b c", b=8, c=8)
                    nc.scalar.activation(m1v[:, :, 1:9, 1:9], p1v, Relu)
                    p2 = ps2.tile([128, 448], f32)
                    for tap in range(9):
                        dy, dx = tap // 3, tap % 3
                        nc.tensor.matmul(
                            p2[:],
                            w2t[:, (pair * 9 + tap) * 128:(pair * 9 + tap + 1) * 128],
                            m1v[:, :, dy:dy + 8, dx:dx + 8],
                            start=(tap == 0), stop=(tap == 8))
                    s["p2"] = p2
                # ---- stage C (job i-2): m2relu, conv3, residual, store ----
                if 0 <= i - 2 < len(jobs):
                    s = state.pop(i - 2)
                    b, pair, py = s["b"], s["pair"], s["py"]
                    xviews = s["xviews"]
                    mseg = mkt[:, (b * 2 + pair) * 49 + py * 7:
                               (b * 2 + pair) * 49 + py * 7 + 7]
                    mbc = mseg.unsqueeze(2).broadcast_to([128, 7, 64])
                    m2 = m2pool.tile([128, 448], bf16)
                    m2v = m2[:].rearrange("p (a b) -> p a b", a=7)
                    p2v3 = s["p2"][:].rearrange("p (a b) -> p a b", a=7)
                    nc.vector.scalar_tensor_tensor(m2v, p2v3, 0.0, mbc, Max, Mult)
                    m2r = m2[:]
                    ot = opool.tile([128, 4 * 448], bf16)
                    for mt in range(2):
                        for gi in range(2):
                            blk = (pair * 2 + gi) * 2 + mt
                            ct = 2 * gi + mt
                            p3 = ps3.tile([128, 448], f32)
                            nc.tensor.matmul(
                                p3[:],
                                w3t[64 * gi:64 * gi + 64, blk * 128:(blk + 1) * 128],
                                m2r[64 * gi:64 * gi + 64, :])
                            t3 = t3pool.tile([128, 448], bf16)
                            t3v = t3[:].rearrange("p (a b c) -> p a b c", b=8, c=8)
                            p3v = p3[:].rearrange("p (a b c) -> p a b c", b=8, c=8)
                            nc.vector.tensor_add(t3v, p3v, xviews[ct])
                            otv = ot[:, ct * 448:(ct + 1) * 448].rearrange(
                                "p (y px x) -> p px y x", y=8, px=7, x=8)
                            if ct < 2:
                                nc.scalar.activation(otv, t3v, Relu)
                            else:
                                nc.gpsimd.tensor_scalar_max(otv, t3v, 0.0)
                    store_eng.dma_start(
                        oap[b, 512 * pair:512 * pair + 512,
                            8 * py:8 * py + 8, :]
                        .rearrange("(u c) h w -> c u (h w)", u=4),
                        ot[:].rearrange("p (u s) -> p u s", u=4))
                    if py == MS - 1:
                        del xts_for_set[s["si"]]
    nc.compile()
    return nc


def _get_program():
    if "nc" not in _CACHE:
        _CACHE["nc"] = _build_program()
    return _CACHE["nc"]


def make_in_maps(x, mask, w1, w2, w3):
    bf16 = _bf16()
    x = np.ascontiguousarray(np.asarray(x, np.float32)).astype(bf16)
    mask = np.asarray(mask, np.float32)
    w1s, w2s, w3s = _pack_weights(np.asarray(w1, np.float32),
                                  np.asarray(w2, np.float32),
                                  np.asarray(w3, np.float32))
    w1s, w2s, w3s = w1s.astype(bf16), w2s.astype(bf16), w3s.astype(bf16)
    in_maps = []
    for k in range(NCORES):
        in_maps.append({
            "x": x[BPC * k:BPC * (k + 1)],
            "maskrep": _pack_mask(mask[BPC * k:BPC * (k + 1)]),
            "w1s": w1s, "w2s": w2s, "w3s": w3s,
        })
    return in_maps


def kernel(x, mask, w1, w2, w3):
    from concourse import bass_utils

    in_maps = make_in_maps(x, mask, w1, w2, w3)
    nc = _get_program()
    res = bass_utils.run_bass_kernel_spmd(nc, in_maps, core_ids=list(range(NCORES)))
    out = np.concatenate([res.results[k]["out"] for k in range(NCORES)], axis=0)
    return out.astype(np.float32)


# revision 4
# speedup vs baseline: 2.4087x; 2.4087x over previous
"""Trainium2 Bass kernel for masked grouped-bottleneck (moe_routing patch refine).

Full computation:
  x [16,1024,56,56] is split into a 7x7 grid of 8x8 patches; per patch a
  grouped (G=4) bottleneck conv1(1x1)->relu->conv2(3x3, per-patch pad)->relu
  ->conv3(1x1) runs; the result is zeroed for non-selected (b, group, patch)
  combos per `mask`, un-patchified, added to x (residual) and relu'd.

Sharding: data-parallel over batch, 2 images per core across 8 cores.

All tensors ride in bf16 (inputs downcast on the host, output upcast back to
fp32 on the host): the 2e-2 rel-err budget dwarfs bf16's ~2e-3, and halving
the HBM bytes moves the DMA floor from ~143us to ~71us per core while the PE
stays at 1 cycle/row either way (fp32r is also 1 cycle/row at N=448).

Weights are repacked on the host into PE-friendly lhsT layouts (block-diagonal
over group pairs so conv2 runs dense K=128/M=128 matmuls). The routing mask is
applied right after conv2: every conv is patch-local and bias-free, so zeroing
m2 for a (group, patch) is exactly equivalent to zeroing the conv3 output.

Pipeline per (batch, group-pair) macro-iteration, streaming 7 patch rows:
  conv1 (PE, 4 accumulating blockdiag matmuls) -> relu (ACT) into the zero-
  padded m1 interior (borders memset once, first 3 jobs only — they are never
  overwritten) -> conv2 as 9 accumulating taps over shifted padded views
  (walrus requires matmul OUT APs to canonicalize to <=3 dims, so clipped
  PSUM sub-views are not an option) -> fused mask*relu (one DVE
  scalar_tensor_tensor m2 = (p2 max 0) * mask) -> conv3 (PE) -> residual add
  (DVE) -> relu into a 4-slab output tile, blocks 0/1 on ACT and blocks 2/3 on
  Pool (gpsimd), keeping both under the PE roofline -> one store per patch row.

Engine budget per core (28 jobs, cost-model): PE 17x448 rows/job = 89us,
DVE (stt + 4 residual adds, PSUM-bound so no 2x mode) = 83us, ACT = 46us,
Pool = 40us, DMA 25.7MB = 71us.

DMA: x loads (one 3.2MB dma_start per set) and the batched stores ride the SP
(sync) HWDGE ring; loads for a set are issued two sets ahead (xpool bufs=3).
Stores must NOT go on the ACT ring: a store's sem wait would hold the ACT
sequencer and block the next row's relu decodes.
"""
import numpy as np

_CACHE = {}

B, C, H, W = 16, 1024, 56, 56
G, MS, HP = 4, 7, 8
MID = 256
NCORES = 8
BPC = B // NCORES   # batches per core


def _bf16():
    from concourse import mybir
    return mybir.dt.np(mybir.dt.bfloat16)


def _pack_weights(w1, w2, w3):
    w1s = np.zeros((128, 2 * 4 * 128), np.float32)
    for pair in range(2):
        for j in range(4):            # K-tile over the pair's 512 input chans
            gi, kt = j // 2, j % 2
            g = 2 * pair + gi
            Wg = w1[64 * g:64 * g + 64, 128 * kt:128 * kt + 128, 0, 0]
            w1s[:, (pair * 4 + j) * 128 + 64 * gi:(pair * 4 + j) * 128 + 64 * gi + 64] = Wg.T
    w2s = np.zeros((128, 2 * 9 * 128), np.float32)
    for pair in range(2):
        for tap in range(9):
            dy, dx = tap // 3, tap % 3
            for gi in range(2):
                g = 2 * pair + gi
                Wg = w2[64 * g:64 * g + 64, :, dy, dx]
                w2s[64 * gi:64 * gi + 64,
                    (pair * 9 + tap) * 128 + 64 * gi:(pair * 9 + tap) * 128 + 64 * gi + 64] = Wg.T
    w3s = np.zeros((128, 8 * 128), np.float32)
    for pair in range(2):
        for gi in range(2):
            g = 2 * pair + gi
            for mt in range(2):
                Wg = w3[256 * g + 128 * mt:256 * g + 128 * (mt + 1), :, 0, 0]
                blk = (pair * 2 + gi) * 2 + mt
                w3s[64 * gi:64 * gi + 64, blk * 128:(blk + 1) * 128] = Wg.T
    return w1s, w2s, w3s


def _pack_mask(mask_b):
    # mask_b: [BPC, 4, 7, 7] -> [128, BPC*2*49], row r belongs to group 2*pair + r//64
    m = np.zeros((128, BPC * 2 * 49), np.float32)
    mb = (mask_b > 0).astype(np.float32).reshape(BPC, 4, 49)
    for b in range(BPC):
        for pair in range(2):
            seg = slice((b * 2 + pair) * 49, (b * 2 + pair + 1) * 49)
            m[0:64, seg] = mb[b, 2 * pair]
            m[64:128, seg] = mb[b, 2 * pair + 1]
    return m


def _build_program(reps=1, store_engine="sync"):
    import concourse.bacc as bacc
    import concourse.mybir as mybir
    import concourse.tile as tile

    f32 = mybir.dt.float32
    bf16 = mybir.dt.bfloat16
    Relu = mybir.ActivationFunctionType.Relu
    Max = mybir.AluOpType.max
    Mult = mybir.AluOpType.mult

    nc = bacc.Bacc("TRN2", target_bir_lowering=False, debug=False)
    x_d = nc.dram_tensor("x", [BPC, C, H, W], bf16, kind="ExternalInput")
    mk_d = nc.dram_tensor("maskrep", [128, BPC * 2 * 49], f32, kind="ExternalInput")
    w1_d = nc.dram_tensor("w1s", [128, 1024], bf16, kind="ExternalInput")
    w2_d = nc.dram_tensor("w2s", [128, 2304], bf16, kind="ExternalInput")
    w3_d = nc.dram_tensor("w3s", [128, 1024], bf16, kind="ExternalInput")
    out_d = nc.dram_tensor("out", [BPC, C, H, W], bf16, kind="ExternalOutput")

    xap = x_d.ap()
    oap = out_d.ap()

    with tile.TileContext(nc) as tc:
        with (
            tc.tile_pool(name="wpool", bufs=1) as wpool,
            tc.tile_pool(name="xpool", bufs=3) as xpool,
            tc.tile_pool(name="m1pool", bufs=3) as m1pool,
            tc.tile_pool(name="m2pool", bufs=3) as m2pool,
            tc.tile_pool(name="t3pool", bufs=3) as t3pool,
            tc.tile_pool(name="opool", bufs=3) as opool,
            tc.tile_pool(name="ps1", bufs=2, space="PSUM") as ps1,
            tc.tile_pool(name="ps2", bufs=2, space="PSUM") as ps2,
            tc.tile_pool(name="ps3", bufs=4, space="PSUM") as ps3,
        ):
            w1t = wpool.tile([128, 1024], bf16, tag="w1")
            w2t = wpool.tile([128, 2304], bf16, tag="w2")
            w3t = wpool.tile([128, 1024], bf16, tag="w3")
            mkt = wpool.tile([128, BPC * 2 * 49], f32, tag="mk")
            # weights go on the ACT HWDGE ring so they don't queue ahead
            # of the first x-tile loads on the sync ring at startup
            nc.scalar.dma_start(w1t[:], w1_d.ap())
            nc.scalar.dma_start(w2t[:], w2_d.ap())
            nc.scalar.dma_start(w3t[:], w3_d.ap())
            nc.scalar.dma_start(mkt[:], mk_d.ap())

            store_eng = nc.scalar if store_engine == "scalar" else nc.sync

            # (b, pair) macro-sets, 7 patch-row jobs each, pipelined 2 deep:
            # iteration i runs conv1[i], relu1+conv2[i-1], and
            # stt+conv3+residual+store[i-2] so no engine queue ever waits
            # on a same-row cross-engine dependency.
            set_list = [(b, pair)
                        for _ in range(reps)
                        for b in range(BPC)
                        for pair in range(2)]
            xts_for_set = {}

            def load_set(si):
                b, pair = set_list[si]
                xt = xpool.tile([128, 4 * H * W], bf16, tag="xt", name=f"xt{si}")
                nc.sync.dma_start(
                    xt[:].rearrange("c (u s) -> c u s", u=4),
                    xap[b, 512 * pair:512 * pair + 512]
                    .rearrange("(u c) h w -> c u (h w)", u=4))
                xts_for_set[si] = xt

            jobs = [(si, py) for si in range(len(set_list)) for py in range(MS)]
            load_set(0)
            if len(set_list) > 1:
                load_set(1)
            state = {}

            for i in range(len(jobs) + 2):
                # ---- stage A (job i): conv1 ----
                if i < len(jobs):
                    si, py = jobs[i]
                    if py == 0 and si + 2 < len(set_list):
                        load_set(si + 2)
                    b, pair = set_list[si]
                    xt = xts_for_set[si]
                    xviews = [
                        xt[:].rearrange("p (u py y px x) -> p u py px y x",
                                        u=4, py=7, y=8, px=7, x=8)[:, u, py]
                        for u in range(4)
                    ]
                    p1 = ps1.tile([128, 448], f32)
                    for j in range(4):
                        nc.tensor.matmul(
                            p1[:],
                            w1t[:, (pair * 4 + j) * 128:(pair * 4 + j + 1) * 128],
                            xviews[j],
                            start=(j == 0), stop=(j == 3))
                    state[i] = dict(si=si, py=py, b=b, pair=pair,
                                    xviews=xviews, p1=p1)
                # ---- stage B (job i-1): relu into padded m1, conv2 ----
                if 0 <= i - 1 < len(jobs):
                    s = state[i - 1]
                    b, pair, py = s["b"], s["pair"], s["py"]
                    m1 = m1pool.tile([128, 700], bf16)
                    if i - 1 < 3:
                        # zero the patch borders once per physical buffer;
                        # only the 8x8 interior is ever rewritten after this
                        nc.gpsimd.memset(m1[:].bitcast(f32), 0.0)
                    m1v = m1[:].rearrange("p (a b c) -> p a b c", b=10, c=10)
                    p1v = s["p1"][:].rearrange("p (a b c) -> p a b c", b=8, c=8)
                    nc.scalar.activation(m1v[:, :, 1:9, 1:9], p1v, Relu)
                    p2 = ps2.tile([128, 448], f32)
                    for tap in range(9):
                        dy, dx = tap // 3, tap % 3
                        nc.tensor.matmul(
                            p2[:],
                            w2t[:, (pair * 9 + tap) * 128:(pair * 9 + tap + 1) * 128],
                            m1v[:, :, dy:dy + 8, dx:dx + 8],
                            start=(tap == 0), stop=(tap == 8))
                    s["p2"] = p2
                # ---- stage C (job i-2): m2relu, conv3, residual, store ----
                if 0 <= i - 2 < len(jobs):
                    s = state.pop(i - 2)
                    b, pair, py = s["b"], s["pair"], s["py"]
                    xviews = s["xviews"]
                    mseg = mkt[:, (b * 2 + pair) * 49 + py * 7:
                               (b * 2 + pair) * 49 + py * 7 + 7]
                    mbc = mseg.unsqueeze(2).broadcast_to([128, 7, 64])
                    m2 = m2pool.tile([128, 448], bf16)
                    m2v = m2[:].rearrange("p (a b) -> p a b", a=7)
                    p2v3 = s["p2"][:].rearrange("p (a b) -> p a b", a=7)
                    nc.vector.scalar_tensor_tensor(m2v, p2v3, 0.0, mbc, Max, Mult)
                    m2r = m2[:]
                    ot = opool.tile([128, 4 * 448], bf16)
                    for mt in range(2):
                        for gi in range(2):
                            blk = (pair * 2 + gi) * 2 + mt
                            ct = 2 * gi + mt
                            p3 = ps3.tile([128, 448], f32)
                            nc.tensor.matmul(
                                p3[:],
                                w3t[64 * gi:64 * gi + 64, blk * 128:(blk + 1) * 128],
                                m2r[64 * gi:64 * gi + 64, :])
                            t3 = t3pool.tile([128, 448], bf16)
                            t3v = t3[:].rearrange("p (a b c) -> p a b c", b=8, c=8)
                            p3v = p3[:].rearrange("p (a b c) -> p a b c", b=8, c=8)
                            nc.vector.tensor_add(t3v, p3v, xviews[ct])
                            otv = ot[:, ct * 448:(ct + 1) * 448].rearrange(
                                "p (y px x) -> p px y x", y=8, px=7, x=8)
                            nc.scalar.activation(otv, t3v, Relu)
                    store_eng.dma_start(
                        oap[b, 512 * pair:512 * pair + 512,
                            8 * py:8 * py + 8, :]
                        .rearrange("(u c) h w -> c u (h w)", u=4),
                        ot[:].rearrange("p (u s) -> p u s", u=4))
                    if py == MS - 1:
                        del xts_for_set[s["si"]]
    nc.compile()
    return nc


def _get_program():
    if "nc" not in _CACHE:
        _CACHE["nc"] = _build_program()
    return _CACHE["nc"]


def make_in_maps(x, mask, w1, w2, w3):
    bf16 = _bf16()
    x = np.ascontiguousarray(np.asarray(x, np.float32)).astype(bf16)
    mask = np.asarray(mask, np.float32)
    w1s, w2s, w3s = _pack_weights(np.asarray(w1, np.float32),
                                  np.asarray(w2, np.float32),
                                  np.asarray(w3, np.float32))
    w1s, w2s, w3s = w1s.astype(bf16), w2s.astype(bf16), w3s.astype(bf16)
    in_maps = []
    for k in range(NCORES):
        in_maps.append({
            "x": x[BPC * k:BPC * (k + 1)],
            "maskrep": _pack_mask(mask[BPC * k:BPC * (k + 1)]),
            "w1s": w1s, "w2s": w2s, "w3s": w3s,
        })
    return in_maps


def kernel(x, mask, w1, w2, w3):
    from concourse import bass_utils

    in_maps = make_in_maps(x, mask, w1, w2, w3)
    nc = _get_program()
    res = bass_utils.run_bass_kernel_spmd(nc, in_maps, core_ids=list(range(NCORES)))
    out = np.concatenate([res.results[k]["out"] for k in range(NCORES)], axis=0)
    return out.astype(np.float32)


# revision 6
# speedup vs baseline: 3.0822x; 1.2796x over previous
"""Trainium2 Bass kernel for masked grouped-bottleneck (moe_routing patch refine).

Full computation:
  x [16,1024,56,56] is split into a 7x7 grid of 8x8 patches; per patch a
  grouped (G=4) bottleneck conv1(1x1)->relu->conv2(3x3, per-patch pad)->relu
  ->conv3(1x1) runs; the result is zeroed for non-selected (b, group, patch)
  combos per `mask`, un-patchified, added to x (residual) and relu'd.

Sharding: data-parallel over batch, 2 images per core across 8 cores.

All tensors ride in bf16 (inputs downcast on the host, output upcast back to
fp32 on the host): the 2e-2 rel-err budget dwarfs bf16's ~2e-3, and halving
the HBM bytes moves the DMA floor from ~143us to ~71us per core while the PE
stays at 1 cycle/row either way (fp32r is also 1 cycle/row at N=448).

Weights are repacked on the host into PE-friendly lhsT layouts (block-diagonal
over group pairs so conv2 runs dense K=128/M=128 matmuls). The routing mask is
applied right after conv2: every conv is patch-local and bias-free, so zeroing
m2 for a (group, patch) is exactly equivalent to zeroing the conv3 output.

Pipeline per (batch, group-pair) macro-iteration, streaming 7 patch rows:
  conv1 (PE, 4 accumulating blockdiag matmuls) -> relu (ACT) into the zero-
  padded m1 interior (borders memset once, first 3 jobs only — they are never
  overwritten) -> conv2 as 9 accumulating taps over shifted padded views
  (walrus requires matmul OUT APs to canonicalize to <=3 dims, so clipped
  PSUM sub-views are not an option) -> fused mask*relu (one DVE
  scalar_tensor_tensor m2 = (p2 max 0) * mask) -> conv3 (PE) -> residual add
  (DVE) -> relu into a 4-slab output tile, blocks 0/1 on ACT and blocks 2/3 on
  Pool (gpsimd), keeping both under the PE roofline -> one store per patch row.

Engine budget per core (28 jobs, cost-model): PE 17x448 rows/job = 89us,
DVE (stt + 4 residual adds, PSUM-bound so no 2x mode) = 83us, ACT = 46us,
Pool = 40us, DMA 25.7MB = 71us.

DMA: x loads (one 3.2MB dma_start per set) and the batched stores ride the SP
(sync) HWDGE ring; loads for a set are issued two sets ahead (xpool bufs=3).
Stores must NOT go on the ACT ring: a store's sem wait would hold the ACT
sequencer and block the next row's relu decodes.
"""
import numpy as np

_CACHE = {}

B, C, H, W = 16, 1024, 56, 56
G, MS, HP = 4, 7, 8
MID = 256
NCORES = 8
BPC = B // NCORES   # batches per core


def _bf16():
    from concourse import mybir
    return mybir.dt.np(mybir.dt.bfloat16)


def _pack_weights(w1, w2, w3):
    w1s = np.zeros((128, 2 * 4 * 128), np.float32)
    for pair in range(2):
        for j in range(4):            # K-tile over the pair's 512 input chans
            gi, kt = j // 2, j % 2
            g = 2 * pair + gi
            Wg = w1[64 * g:64 * g + 64, 128 * kt:128 * kt + 128, 0, 0]
            w1s[:, (pair * 4 + j) * 128 + 64 * gi:(pair * 4 + j) * 128 + 64 * gi + 64] = Wg.T
    w2s = np.zeros((128, 2 * 9 * 128), np.float32)
    for pair in range(2):
        for tap in range(9):
            dy, dx = tap // 3, tap % 3
            for gi in range(2):
                g = 2 * pair + gi
                Wg = w2[64 * g:64 * g + 64, :, dy, dx]
                w2s[64 * gi:64 * gi + 64,
                    (pair * 9 + tap) * 128 + 64 * gi:(pair * 9 + tap) * 128 + 64 * gi + 64] = Wg.T
    w3s = np.zeros((128, 8 * 128), np.float32)
    for pair in range(2):
        for gi in range(2):
            g = 2 * pair + gi
            for mt in range(2):
                Wg = w3[256 * g + 128 * mt:256 * g + 128 * (mt + 1), :, 0, 0]
                blk = (pair * 2 + gi) * 2 + mt
                w3s[64 * gi:64 * gi + 64, blk * 128:(blk + 1) * 128] = Wg.T
    return w1s, w2s, w3s


def _pack_mask(mask_b):
    # mask_b: [BPC, 4, 7, 7] -> [128, BPC*2*49], row r belongs to group 2*pair + r//64
    m = np.zeros((128, BPC * 2 * 49), np.float32)
    mb = (mask_b > 0).astype(np.float32).reshape(BPC, 4, 49)
    for b in range(BPC):
        for pair in range(2):
            seg = slice((b * 2 + pair) * 49, (b * 2 + pair + 1) * 49)
            m[0:64, seg] = mb[b, 2 * pair]
            m[64:128, seg] = mb[b, 2 * pair + 1]
    return m


def _build_program(reps=1, store_engine="pool"):
    import concourse.bacc as bacc
    import concourse.mybir as mybir
    import concourse.tile as tile

    f32 = mybir.dt.float32
    bf16 = mybir.dt.bfloat16
    Relu = mybir.ActivationFunctionType.Relu
    Max = mybir.AluOpType.max
    Mult = mybir.AluOpType.mult

    nc = bacc.Bacc("TRN2", target_bir_lowering=False, debug=False)
    x_d = nc.dram_tensor("x", [BPC, C, H, W], bf16, kind="ExternalInput")
    mk_d = nc.dram_tensor("maskrep", [128, BPC * 2 * 49], f32, kind="ExternalInput")
    w1_d = nc.dram_tensor("w1s", [128, 1024], bf16, kind="ExternalInput")
    w2_d = nc.dram_tensor("w2s", [128, 2304], bf16, kind="ExternalInput")
    w3_d = nc.dram_tensor("w3s", [128, 1024], bf16, kind="ExternalInput")
    out_d = nc.dram_tensor("out", [BPC, C, H, W], bf16, kind="ExternalOutput")

    xap = x_d.ap()
    oap = out_d.ap()

    with tile.TileContext(nc) as tc:
        with (
            tc.tile_pool(name="wpool", bufs=1) as wpool,
            tc.tile_pool(name="xpool", bufs=3) as xpool,
            tc.tile_pool(name="m1pool", bufs=3) as m1pool,
            tc.tile_pool(name="m2pool", bufs=3) as m2pool,
            tc.tile_pool(name="t3pool", bufs=3) as t3pool,
            tc.tile_pool(name="opool", bufs=3) as opool,
            tc.tile_pool(name="ps1", bufs=2, space="PSUM") as ps1,
            tc.tile_pool(name="ps2", bufs=2, space="PSUM") as ps2,
            tc.tile_pool(name="ps3", bufs=4, space="PSUM") as ps3,
        ):
            w1t = wpool.tile([128, 1024], bf16, tag="w1")
            w2t = wpool.tile([128, 2304], bf16, tag="w2")
            w3t = wpool.tile([128, 1024], bf16, tag="w3")
            mkt = wpool.tile([128, BPC * 2 * 49], f32, tag="mk")
            # weights go on the ACT HWDGE ring so they don't queue ahead
            # of the first x-tile loads on the sync ring at startup
            nc.scalar.dma_start(w1t[:], w1_d.ap())
            nc.scalar.dma_start(w2t[:], w2_d.ap())
            nc.scalar.dma_start(w3t[:], w3_d.ap())
            nc.scalar.dma_start(mkt[:], mk_d.ap())

            store_eng = {"scalar": nc.scalar, "pool": nc.gpsimd,
                         "sync": nc.sync}[store_engine]

            # (b, pair) macro-sets, 7 patch-row jobs each, pipelined 2 deep:
            # iteration i runs conv1[i], relu1+conv2[i-1], and
            # stt+conv3+residual+store[i-2] so no engine queue ever waits
            # on a same-row cross-engine dependency.
            set_list = [(b, pair)
                        for _ in range(reps)
                        for b in range(BPC)
                        for pair in range(2)]
            xts_for_set = {}

            def load_set(si):
                b, pair = set_list[si]
                xt = xpool.tile([128, 4 * H * W], bf16, tag="xt", name=f"xt{si}")
                nc.sync.dma_start(
                    xt[:].rearrange("c (u s) -> c u s", u=4),
                    xap[b, 512 * pair:512 * pair + 512]
                    .rearrange("(u c) h w -> c u (h w)", u=4))
                xts_for_set[si] = xt

            jobs = [(si, py) for si in range(len(set_list)) for py in range(MS)]
            load_set(0)
            if len(set_list) > 1:
                load_set(1)
            state = {}

            for i in range(len(jobs) + 2):
                # ---- stage A (job i): conv1 ----
                if i < len(jobs):
                    si, py = jobs[i]
                    if py == 0 and si + 2 < len(set_list):
                        load_set(si + 2)
                    b, pair = set_list[si]
                    xt = xts_for_set[si]
                    xviews = [
                        xt[:].rearrange("p (u py y px x) -> p u py px y x",
                                        u=4, py=7, y=8, px=7, x=8)[:, u, py]
                        for u in range(4)
                    ]
                    p1 = ps1.tile([128, 448], f32)
                    for j in range(4):
                        nc.tensor.matmul(
                            p1[:],
                            w1t[:, (pair * 4 + j) * 128:(pair * 4 + j + 1) * 128],
                            xviews[j],
                            start=(j == 0), stop=(j == 3))
                    state[i] = dict(si=si, py=py, b=b, pair=pair,
                                    xviews=xviews, p1=p1)
                # ---- stage B (job i-1): relu into padded m1, conv2 ----
                if 0 <= i - 1 < len(jobs):
                    s = state[i - 1]
                    b, pair, py = s["b"], s["pair"], s["py"]
                    m1 = m1pool.tile([128, 700], bf16)
                    if i - 1 < 3:
                        # zero the patch borders once per physical buffer;
                        # only the 8x8 interior is ever rewritten after this
                        nc.gpsimd.memset(m1[:].bitcast(f32), 0.0)
                    m1v = m1[:].rearrange("p (a b c) -> p a b c", b=10, c=10)
                    p1v = s["p1"][:].rearrange("p (a b c) -> p a b c", b=8, c=8)
                    nc.scalar.activation(m1v[:, :, 1:9, 1:9], p1v, Relu)
                    p2 = ps2.tile([128, 448], f32)
                    for tap in range(9):
                        dy, dx = tap // 3, tap % 3
                        nc.tensor.matmul(
                            p2[:],
                            w2t[:, (pair * 9 + tap) * 128:(pair * 9 + tap + 1) * 128],
                            m1v[:, :, dy:dy + 8, dx:dx + 8],
                            start=(tap == 0), stop=(tap == 8))
                    s["p2"] = p2
                # ---- stage C (job i-2): m2relu, conv3, residual, store ----
                if 0 <= i - 2 < len(jobs):
                    s = state.pop(i - 2)
                    b, pair, py = s["b"], s["pair"], s["py"]
                    xviews = s["xviews"]
                    mseg = mkt[:, (b * 2 + pair) * 49 + py * 7:
                               (b * 2 + pair) * 49 + py * 7 + 7]
                    mbc = mseg.unsqueeze(2).broadcast_to([128, 7, 64])
                    m2 = m2pool.tile([128, 448], bf16)
                    m2v = m2[:].rearrange("p (a b) -> p a b", a=7)
                    p2v3 = s["p2"][:].rearrange("p (a b) -> p a b", a=7)
                    nc.vector.scalar_tensor_tensor(m2v, p2v3, 0.0, mbc, Max, Mult)
                    m2r = m2[:]
                    ot = opool.tile([128, 4 * 448], bf16)
                    for mt in range(2):
                        for gi in range(2):
                            blk = (pair * 2 + gi) * 2 + mt
                            ct = 2 * gi + mt
                            p3 = ps3.tile([128, 448], f32)
                            nc.tensor.matmul(
                                p3[:],
                                w3t[64 * gi:64 * gi + 64, blk * 128:(blk + 1) * 128],
                                m2r[64 * gi:64 * gi + 64, :])
                            t3 = t3pool.tile([128, 448], bf16)
                            t3v = t3[:].rearrange("p (a b c) -> p a b c", b=8, c=8)
                            p3v = p3[:].rearrange("p (a b c) -> p a b c", b=8, c=8)
                            nc.vector.tensor_add(t3v, p3v, xviews[ct])
                            otv = ot[:, ct * 448:(ct + 1) * 448].rearrange(
                                "p (y px x) -> p px y x", y=8, px=7, x=8)
                            nc.scalar.activation(otv, t3v, Relu)
                    store_eng.dma_start(
                        oap[b, 512 * pair:512 * pair + 512,
                            8 * py:8 * py + 8, :]
                        .rearrange("(u c) h w -> c u (h w)", u=4),
                        ot[:].rearrange("p (u s) -> p u s", u=4))
                    if py == MS - 1:
                        del xts_for_set[s["si"]]
    nc.compile()
    return nc


def _get_program():
    if "nc" not in _CACHE:
        _CACHE["nc"] = _build_program()
    return _CACHE["nc"]


def make_in_maps(x, mask, w1, w2, w3):
    bf16 = _bf16()
    x = np.ascontiguousarray(np.asarray(x, np.float32)).astype(bf16)
    mask = np.asarray(mask, np.float32)
    w1s, w2s, w3s = _pack_weights(np.asarray(w1, np.float32),
                                  np.asarray(w2, np.float32),
                                  np.asarray(w3, np.float32))
    w1s, w2s, w3s = w1s.astype(bf16), w2s.astype(bf16), w3s.astype(bf16)
    in_maps = []
    for k in range(NCORES):
        in_maps.append({
            "x": x[BPC * k:BPC * (k + 1)],
            "maskrep": _pack_mask(mask[BPC * k:BPC * (k + 1)]),
            "w1s": w1s, "w2s": w2s, "w3s": w3s,
        })
    return in_maps


def kernel(x, mask, w1, w2, w3):
    from concourse import bass_utils

    in_maps = make_in_maps(x, mask, w1, w2, w3)
    nc = _get_program()
    res = bass_utils.run_bass_kernel_spmd(nc, in_maps, core_ids=list(range(NCORES)))
    out = np.concatenate([res.results[k]["out"] for k in range(NCORES)], axis=0)
    return out.astype(np.float32)


# revision 8
# speedup vs baseline: 3.5546x; 1.1533x over previous
"""Trainium2 Bass kernel for masked grouped-bottleneck (moe_routing patch refine).

Full computation:
  x [16,1024,56,56] is split into a 7x7 grid of 8x8 patches; per patch a
  grouped (G=4) bottleneck conv1(1x1)->relu->conv2(3x3, per-patch pad)->relu
  ->conv3(1x1) runs; the result is zeroed for non-selected (b, group, patch)
  combos per `mask`, un-patchified, added to x (residual) and relu'd.

Sharding: data-parallel over batch, 2 images per core across 8 cores.

All tensors ride in bf16 (inputs downcast on the host, output upcast back to
fp32 on the host): the 2e-2 rel-err budget dwarfs bf16's ~2e-3, and halving
the HBM bytes moves the DMA floor from ~143us to ~71us per core while the PE
stays at 1 cycle/row either way (fp32r is also 1 cycle/row at N=448).

Weights are repacked on the host into PE-friendly lhsT layouts (block-diagonal
over group pairs so conv2 runs dense K=128/M=128 matmuls). The routing mask is
applied right after conv2: every conv is patch-local and bias-free, so zeroing
m2 for a (group, patch) is exactly equivalent to zeroing the conv3 output.

Pipeline per (batch, group-pair) macro-iteration, streaming 7 patch rows:
  conv1 (PE, 4 accumulating blockdiag matmuls) -> relu (ACT) into the zero-
  padded m1 interior (borders memset once, first 3 jobs only — they are never
  overwritten) -> conv2 as 9 accumulating taps over shifted padded views
  (walrus requires matmul OUT APs to canonicalize to <=3 dims, so clipped
  PSUM sub-views are not an option) -> fused mask*relu (one DVE
  scalar_tensor_tensor m2 = (p2 max 0) * mask) -> conv3 (PE) -> residual add
  (DVE) -> relu into a 4-slab output tile, blocks 0/1 on ACT and blocks 2/3 on
  Pool (gpsimd), keeping both under the PE roofline -> one store per patch row.

Engine budget per core (28 jobs, cost-model): PE 17x448 rows/job = 89us,
DVE (stt + 4 residual adds, PSUM-bound so no 2x mode) = 83us, ACT = 46us,
Pool = 40us, DMA 25.7MB = 71us.

DMA: x loads (one 3.2MB dma_start per set) and the batched stores ride the SP
(sync) HWDGE ring; loads for a set are issued two sets ahead (xpool bufs=3).
Stores must NOT go on the ACT ring: a store's sem wait would hold the ACT
sequencer and block the next row's relu decodes.
"""
import numpy as np

_CACHE = {}

B, C, H, W = 16, 1024, 56, 56
G, MS, HP = 4, 7, 8
MID = 256
NCORES = 8
BPC = B // NCORES   # batches per core


def _bf16():
    from concourse import mybir
    return mybir.dt.np(mybir.dt.bfloat16)


def _pack_weights(w1, w2, w3):
    w1s = np.zeros((128, 2 * 4 * 128), np.float32)
    for pair in range(2):
        for j in range(4):            # K-tile over the pair's 512 input chans
            gi, kt = j // 2, j % 2
            g = 2 * pair + gi
            Wg = w1[64 * g:64 * g + 64, 128 * kt:128 * kt + 128, 0, 0]
            w1s[:, (pair * 4 + j) * 128 + 64 * gi:(pair * 4 + j) * 128 + 64 * gi + 64] = Wg.T
    w2s = np.zeros((128, 2 * 9 * 128), np.float32)
    for pair in range(2):
        for tap in range(9):
            dy, dx = tap // 3, tap % 3
            for gi in range(2):
                g = 2 * pair + gi
                Wg = w2[64 * g:64 * g + 64, :, dy, dx]
                w2s[64 * gi:64 * gi + 64,
                    (pair * 9 + tap) * 128 + 64 * gi:(pair * 9 + tap) * 128 + 64 * gi + 64] = Wg.T
    w3s = np.zeros((128, 8 * 128), np.float32)
    for pair in range(2):
        for gi in range(2):
            g = 2 * pair + gi
            for mt in range(2):
                Wg = w3[256 * g + 128 * mt:256 * g + 128 * (mt + 1), :, 0, 0]
                blk = (pair * 2 + gi) * 2 + mt
                w3s[64 * gi:64 * gi + 64, blk * 128:(blk + 1) * 128] = Wg.T
    return w1s, w2s, w3s


def _pack_mask(mask_b):
    # mask_b: [BPC, 4, 7, 7] -> [128, BPC*2*49], row r belongs to group 2*pair + r//64
    m = np.zeros((128, BPC * 2 * 49), np.float32)
    mb = (mask_b > 0).astype(np.float32).reshape(BPC, 4, 49)
    for b in range(BPC):
        for pair in range(2):
            seg = slice((b * 2 + pair) * 49, (b * 2 + pair + 1) * 49)
            m[0:64, seg] = mb[b, 2 * pair]
            m[64:128, seg] = mb[b, 2 * pair + 1]
    return m


def _build_program(reps=1, store_engine="pool"):
    import concourse.bacc as bacc
    import concourse.mybir as mybir
    import concourse.tile as tile

    f32 = mybir.dt.float32
    bf16 = mybir.dt.bfloat16
    Relu = mybir.ActivationFunctionType.Relu
    Max = mybir.AluOpType.max
    Mult = mybir.AluOpType.mult

    nc = bacc.Bacc("TRN2", target_bir_lowering=False, debug=False)
    x_d = nc.dram_tensor("x", [BPC, C, H, W], bf16, kind="ExternalInput")
    mk_d = nc.dram_tensor("maskrep", [128, BPC * 2 * 49], f32, kind="ExternalInput")
    w1_d = nc.dram_tensor("w1s", [128, 1024], bf16, kind="ExternalInput")
    w2_d = nc.dram_tensor("w2s", [128, 2304], bf16, kind="ExternalInput")
    w3_d = nc.dram_tensor("w3s", [128, 1024], bf16, kind="ExternalInput")
    out_d = nc.dram_tensor("out", [BPC, C, H, W], bf16, kind="ExternalOutput")

    xap = x_d.ap()
    oap = out_d.ap()

    with tile.TileContext(nc) as tc:
        with (
            tc.tile_pool(name="wpool", bufs=1) as wpool,
            tc.tile_pool(name="xpool", bufs=3) as xpool,
            tc.tile_pool(name="m1pool", bufs=3) as m1pool,
            tc.tile_pool(name="m2pool", bufs=3) as m2pool,
            tc.tile_pool(name="t3pool", bufs=3) as t3pool,
            tc.tile_pool(name="opool", bufs=3) as opool,
            tc.tile_pool(name="ps1", bufs=2, space="PSUM") as ps1,
            tc.tile_pool(name="ps2", bufs=2, space="PSUM") as ps2,
            tc.tile_pool(name="ps3", bufs=4, space="PSUM") as ps3,
        ):
            w1t = wpool.tile([128, 1024], bf16, tag="w1")
            w2t = wpool.tile([128, 2304], bf16, tag="w2")
            w3t = wpool.tile([128, 1024], bf16, tag="w3")
            mkt = wpool.tile([128, BPC * 2 * 49], f32, tag="mk")
            # weights go on the ACT HWDGE ring so they don't queue ahead
            # of the first x-tile loads on the sync ring at startup
            nc.scalar.dma_start(w1t[:], w1_d.ap())
            nc.scalar.dma_start(w2t[:], w2_d.ap())
            nc.scalar.dma_start(w3t[:], w3_d.ap())
            nc.scalar.dma_start(mkt[:], mk_d.ap())

            store_eng = {"scalar": nc.scalar, "pool": nc.gpsimd,
                         "sync": nc.sync}[store_engine]

            # (b, pair) macro-sets, 7 patch-row jobs each, pipelined 2 deep:
            # iteration i runs conv1[i], relu1+conv2[i-1], and
            # stt+conv3+residual+store[i-2] so no engine queue ever waits
            # on a same-row cross-engine dependency.
            set_list = [(b, pair)
                        for _ in range(reps)
                        for b in range(BPC)
                        for pair in range(2)]
            xts_for_set = {}

            def load_set(si):
                b, pair = set_list[si]
                xts = [xpool.tile([128, H * W], bf16, tag=f"x{u}",
                                  name=f"xt{si}_{u}") for u in range(4)]
                for u in range(4):
                    c0 = 512 * pair + 128 * u
                    nc.sync.dma_start(
                        xts[u][:],
                        xap[b, c0:c0 + 128].rearrange("c h w -> c (h w)"))
                xts_for_set[si] = xts

            jobs = [(si, py) for si in range(len(set_list)) for py in range(MS)]
            load_set(0)
            if len(set_list) > 1:
                load_set(1)
            state = {}

            for i in range(len(jobs) + 2):
                # ---- stage A (job i): conv1 ----
                if i < len(jobs):
                    si, py = jobs[i]
                    if py == 0 and si + 2 < len(set_list):
                        load_set(si + 2)
                    b, pair = set_list[si]
                    xviews = [
                        t[:].rearrange("p (py y px x) -> p py px y x",
                                       py=7, y=8, px=7, x=8)[:, py]
                        for t in xts_for_set[si]
                    ]
                    p1 = ps1.tile([128, 448], f32)
                    for j in range(4):
                        nc.tensor.matmul(
                            p1[:],
                            w1t[:, (pair * 4 + j) * 128:(pair * 4 + j + 1) * 128],
                            xviews[j],
                            start=(j == 0), stop=(j == 3))
                    state[i] = dict(si=si, py=py, b=b, pair=pair,
                                    xviews=xviews, p1=p1)
                # ---- stage B (job i-1): relu into padded m1, conv2 ----
                if 0 <= i - 1 < len(jobs):
                    s = state[i - 1]
                    b, pair, py = s["b"], s["pair"], s["py"]
                    m1 = m1pool.tile([128, 700], bf16)
                    if i - 1 < 3:
                        # zero the patch borders once per physical buffer;
                        # only the 8x8 interior is ever rewritten after this
                        nc.gpsimd.memset(m1[:].bitcast(f32), 0.0)
                    m1v = m1[:].rearrange("p (a b c) -> p a b c", b=10, c=10)
                    p1v = s["p1"][:].rearrange("p (a b c) -> p a b c", b=8, c=8)
                    nc.scalar.activation(m1v[:, :, 1:9, 1:9], p1v, Relu)
                    p2 = ps2.tile([128, 448], f32)
                    for tap in range(9):
                        dy, dx = tap // 3, tap % 3
                        nc.tensor.matmul(
                            p2[:],
                            w2t[:, (pair * 9 + tap) * 128:(pair * 9 + tap + 1) * 128],
                            m1v[:, :, dy:dy + 8, dx:dx + 8],
                            start=(tap == 0), stop=(tap == 8))
                    s["p2"] = p2
                # ---- stage C (job i-2): m2relu, conv3, residual, store ----
                if 0 <= i - 2 < len(jobs):
                    s = state.pop(i - 2)
                    b, pair, py = s["b"], s["pair"], s["py"]
                    xviews = s["xviews"]
                    mseg = mkt[:, (b * 2 + pair) * 49 + py * 7:
                               (b * 2 + pair) * 49 + py * 7 + 7]
                    mbc = mseg.unsqueeze(2).broadcast_to([128, 7, 64])
                    m2 = m2pool.tile([128, 448], bf16)
                    m2v = m2[:].rearrange("p (a b) -> p a b", a=7)
                    p2v3 = s["p2"][:].rearrange("p (a b) -> p a b", a=7)
                    nc.vector.scalar_tensor_tensor(m2v, p2v3, 0.0, mbc, Max, Mult)
                    m2r = m2[:]
                    ot = opool.tile([128, 4 * 448], bf16)
                    for mt in range(2):
                        for gi in range(2):
                            blk = (pair * 2 + gi) * 2 + mt
                            ct = 2 * gi + mt
                            p3 = ps3.tile([128, 448], f32)
                            nc.tensor.matmul(
                                p3[:],
                                w3t[64 * gi:64 * gi + 64, blk * 128:(blk + 1) * 128],
                                m2r[64 * gi:64 * gi + 64, :])
                            t3 = t3pool.tile([128, 448], bf16)
                            t3v = t3[:].rearrange("p (a b c) -> p a b c", b=8, c=8)
                            p3v = p3[:].rearrange("p (a b c) -> p a b c", b=8, c=8)
                            nc.vector.tensor_add(t3v, p3v, xviews[ct])
                            otv = ot[:, ct * 448:(ct + 1) * 448].rearrange(
                                "p (y px x) -> p px y x", y=8, px=7, x=8)
                            nc.scalar.activation(otv, t3v, Relu)
                    store_eng.dma_start(
                        oap[b, 512 * pair:512 * pair + 512,
                            8 * py:8 * py + 8, :]
                        .rearrange("(u c) h w -> c u (h w)", u=4),
                        ot[:].rearrange("p (u s) -> p u s", u=4))
                    if py == MS - 1:
                        del xts_for_set[s["si"]]
    nc.compile()
    return nc


def _get_program():
    if "nc" not in _CACHE:
        _CACHE["nc"] = _build_program()
    return _CACHE["nc"]


def make_in_maps(x, mask, w1, w2, w3):
    bf16 = _bf16()
    x = np.ascontiguousarray(np.asarray(x, np.float32)).astype(bf16)
    mask = np.asarray(mask, np.float32)
    w1s, w2s, w3s = _pack_weights(np.asarray(w1, np.float32),
                                  np.asarray(w2, np.float32),
                                  np.asarray(w3, np.float32))
    w1s, w2s, w3s = w1s.astype(bf16), w2s.astype(bf16), w3s.astype(bf16)
    in_maps = []
    for k in range(NCORES):
        in_maps.append({
            "x": x[BPC * k:BPC * (k + 1)],
            "maskrep": _pack_mask(mask[BPC * k:BPC * (k + 1)]),
            "w1s": w1s, "w2s": w2s, "w3s": w3s,
        })
    return in_maps


def kernel(x, mask, w1, w2, w3):
    from concourse import bass_utils

    in_maps = make_in_maps(x, mask, w1, w2, w3)
    nc = _get_program()
    res = bass_utils.run_bass_kernel_spmd(nc, in_maps, core_ids=list(range(NCORES)))
    out = np.concatenate([res.results[k]["out"] for k in range(NCORES)], axis=0)
    return out.astype(np.float32)


# revision 9
# speedup vs baseline: 3.6880x; 1.0375x over previous
"""Trainium2 Bass kernel for masked grouped-bottleneck (moe_routing patch refine).

Full computation:
  x [16,1024,56,56] is split into a 7x7 grid of 8x8 patches; per patch a
  grouped (G=4) bottleneck conv1(1x1)->relu->conv2(3x3, per-patch pad)->relu
  ->conv3(1x1) runs; the result is zeroed for non-selected (b, group, patch)
  combos per `mask`, un-patchified, added to x (residual) and relu'd.

Sharding: data-parallel over batch, 2 images per core across 8 cores.

All tensors ride in bf16 (inputs downcast on the host, output upcast back to
fp32 on the host): the 2e-2 rel-err budget dwarfs bf16's ~2e-3, and halving
the HBM bytes moves the DMA floor from ~143us to ~71us per core while the PE
stays at 1 cycle/row either way (fp32r is also 1 cycle/row at N=448).

Weights are repacked on the host into PE-friendly lhsT layouts (block-diagonal
over group pairs so conv2 runs dense K=128/M=128 matmuls). The routing mask is
applied right after conv2: every conv is patch-local and bias-free, so zeroing
m2 for a (group, patch) is exactly equivalent to zeroing the conv3 output.

Pipeline per (batch, group-pair) macro-iteration, streaming 7 patch rows:
  conv1 (PE, 4 accumulating blockdiag matmuls) -> relu (ACT) into the zero-
  padded m1 interior (borders memset once, first 3 jobs only — they are never
  overwritten) -> conv2 as 9 accumulating taps over shifted padded views
  (walrus requires matmul OUT APs to canonicalize to <=3 dims, so clipped
  PSUM sub-views are not an option) -> fused mask*relu (one DVE
  scalar_tensor_tensor m2 = (p2 max 0) * mask) -> conv3 (PE) -> residual add
  (DVE) -> relu into a 4-slab output tile, blocks 0/1 on ACT and blocks 2/3 on
  Pool (gpsimd), keeping both under the PE roofline -> one store per patch row.

Engine budget per core (28 jobs, cost-model): PE 17x448 rows/job = 89us,
DVE (stt + 4 residual adds, PSUM-bound so no 2x mode) = 83us, ACT = 46us,
Pool = 40us, DMA 25.7MB = 71us.

DMA: x loads (one 3.2MB dma_start per set) and the batched stores ride the SP
(sync) HWDGE ring; loads for a set are issued two sets ahead (xpool bufs=3).
Stores must NOT go on the ACT ring: a store's sem wait would hold the ACT
sequencer and block the next row's relu decodes.
"""
import numpy as np

_CACHE = {}

B, C, H, W = 16, 1024, 56, 56
G, MS, HP = 4, 7, 8
MID = 256
NCORES = 8
BPC = B // NCORES   # batches per core


def _bf16():
    from concourse import mybir
    return mybir.dt.np(mybir.dt.bfloat16)


def _pack_weights(w1, w2, w3):
    w1s = np.zeros((128, 2 * 4 * 128), np.float32)
    for pair in range(2):
        for j in range(4):            # K-tile over the pair's 512 input chans
            gi, kt = j // 2, j % 2
            g = 2 * pair + gi
            Wg = w1[64 * g:64 * g + 64, 128 * kt:128 * kt + 128, 0, 0]
            w1s[:, (pair * 4 + j) * 128 + 64 * gi:(pair * 4 + j) * 128 + 64 * gi + 64] = Wg.T
    w2s = np.zeros((128, 2 * 9 * 128), np.float32)
    for pair in range(2):
        for tap in range(9):
            dy, dx = tap // 3, tap % 3
            for gi in range(2):
                g = 2 * pair + gi
                Wg = w2[64 * g:64 * g + 64, :, dy, dx]
                w2s[64 * gi:64 * gi + 64,
                    (pair * 9 + tap) * 128 + 64 * gi:(pair * 9 + tap) * 128 + 64 * gi + 64] = Wg.T
    w3s = np.zeros((128, 8 * 128), np.float32)
    for pair in range(2):
        for gi in range(2):
            g = 2 * pair + gi
            for mt in range(2):
                Wg = w3[256 * g + 128 * mt:256 * g + 128 * (mt + 1), :, 0, 0]
                blk = (pair * 2 + gi) * 2 + mt
                w3s[64 * gi:64 * gi + 64, blk * 128:(blk + 1) * 128] = Wg.T
    return w1s, w2s, w3s


def _pack_mask(mask_b):
    # mask_b: [BPC, 4, 7, 7] -> [128, BPC*2*49], row r belongs to group 2*pair + r//64
    m = np.zeros((128, BPC * 2 * 49), np.float32)
    mb = (mask_b > 0).astype(np.float32).reshape(BPC, 4, 49)
    for b in range(BPC):
        for pair in range(2):
            seg = slice((b * 2 + pair) * 49, (b * 2 + pair + 1) * 49)
            m[0:64, seg] = mb[b, 2 * pair]
            m[64:128, seg] = mb[b, 2 * pair + 1]
    return m


def _build_program(reps=1, store_engine="pool"):
    import concourse.bacc as bacc
    import concourse.mybir as mybir
    import concourse.tile as tile

    f32 = mybir.dt.float32
    bf16 = mybir.dt.bfloat16
    Relu = mybir.ActivationFunctionType.Relu
    Max = mybir.AluOpType.max
    Mult = mybir.AluOpType.mult

    nc = bacc.Bacc("TRN2", target_bir_lowering=False, debug=False)
    x_d = nc.dram_tensor("x", [BPC, C, H, W], bf16, kind="ExternalInput")
    mk_d = nc.dram_tensor("maskrep", [128, BPC * 2 * 49], f32, kind="ExternalInput")
    w1_d = nc.dram_tensor("w1s", [128, 1024], bf16, kind="ExternalInput")
    w2_d = nc.dram_tensor("w2s", [128, 2304], bf16, kind="ExternalInput")
    w3_d = nc.dram_tensor("w3s", [128, 1024], bf16, kind="ExternalInput")
    out_d = nc.dram_tensor("out", [BPC, C, H, W], bf16, kind="ExternalOutput")

    xap = x_d.ap()
    oap = out_d.ap()

    with tile.TileContext(nc) as tc:
        with (
            tc.tile_pool(name="wpool", bufs=1) as wpool,
            tc.tile_pool(name="xpool", bufs=3) as xpool,
            tc.tile_pool(name="m1pool", bufs=3) as m1pool,
            tc.tile_pool(name="m2pool", bufs=3) as m2pool,
            tc.tile_pool(name="t3pool", bufs=3) as t3pool,
            tc.tile_pool(name="opool", bufs=3) as opool,
            tc.tile_pool(name="ps1", bufs=2, space="PSUM") as ps1,
            tc.tile_pool(name="ps2", bufs=2, space="PSUM") as ps2,
            tc.tile_pool(name="ps3", bufs=4, space="PSUM") as ps3,
        ):
            w1t = wpool.tile([128, 1024], bf16, tag="w1")
            w2t = wpool.tile([128, 2304], bf16, tag="w2")
            w3t = wpool.tile([128, 1024], bf16, tag="w3")
            mkt = wpool.tile([128, BPC * 2 * 49], f32, tag="mk")
            # weights go on the ACT HWDGE ring so they don't queue ahead
            # of the first x-tile loads on the sync ring at startup
            nc.scalar.dma_start(w1t[:], w1_d.ap())
            nc.scalar.dma_start(w2t[:], w2_d.ap())
            nc.scalar.dma_start(w3t[:], w3_d.ap())
            nc.scalar.dma_start(mkt[:], mk_d.ap())

            store_eng = {"scalar": nc.scalar, "pool": nc.gpsimd,
                         "sync": nc.sync}[store_engine]

            # (b, pair) macro-sets, 7 patch-row jobs each, pipelined 2 deep:
            # iteration i runs conv1[i], relu1+conv2[i-1], and
            # stt+conv3+residual+store[i-2] so no engine queue ever waits
            # on a same-row cross-engine dependency.
            set_list = [(b, pair)
                        for _ in range(reps)
                        for b in range(BPC)
                        for pair in range(2)]
            xts_for_set = {}

            def load_set(si):
                b, pair = set_list[si]
                xt = xpool.tile([128, 4 * H * W], bf16, tag="xt", name=f"xt{si}")
                nc.sync.dma_start(
                    xt[:].rearrange("c (u s) -> c u s", u=4),
                    xap[b, 512 * pair:512 * pair + 512]
                    .rearrange("(u c) h w -> c u (h w)", u=4))
                xts_for_set[si] = xt

            jobs = [(si, py) for si in range(len(set_list)) for py in range(MS)]
            load_set(0)
            if len(set_list) > 1:
                load_set(1)
            state = {}

            for i in range(len(jobs) + 2):
                # ---- stage A (job i): conv1 ----
                if i < len(jobs):
                    si, py = jobs[i]
                    if py == 0 and si + 2 < len(set_list):
                        load_set(si + 2)
                    b, pair = set_list[si]
                    xt = xts_for_set[si]
                    xviews = [
                        xt[:].rearrange("p (u py y px x) -> p u py px y x",
                                        u=4, py=7, y=8, px=7, x=8)[:, u, py]
                        for u in range(4)
                    ]
                    p1 = ps1.tile([128, 448], f32)
                    for j in range(4):
                        nc.tensor.matmul(
                            p1[:],
                            w1t[:, (pair * 4 + j) * 128:(pair * 4 + j + 1) * 128],
                            xviews[j],
                            start=(j == 0), stop=(j == 3))
                    state[i] = dict(si=si, py=py, b=b, pair=pair,
                                    xviews=xviews, p1=p1)
                # ---- stage B (job i-1): relu into padded m1, conv2 ----
                if 0 <= i - 1 < len(jobs):
                    s = state[i - 1]
                    b, pair, py = s["b"], s["pair"], s["py"]
                    m1 = m1pool.tile([128, 700], bf16)
                    if i - 1 < 3:
                        # zero the patch borders once per physical buffer;
                        # only the 8x8 interior is ever rewritten after this
                        nc.gpsimd.memset(m1[:].bitcast(f32), 0.0)
                    m1v = m1[:].rearrange("p (a b c) -> p a b c", b=10, c=10)
                    p1v = s["p1"][:].rearrange("p (a b c) -> p a b c", b=8, c=8)
                    nc.scalar.activation(m1v[:, :, 1:9, 1:9], p1v, Relu)
                    p2 = ps2.tile([128, 448], f32)
                    for tap in range(9):
                        dy, dx = tap // 3, tap % 3
                        nc.tensor.matmul(
                            p2[:],
                            w2t[:, (pair * 9 + tap) * 128:(pair * 9 + tap + 1) * 128],
                            m1v[:, :, dy:dy + 8, dx:dx + 8],
                            start=(tap == 0), stop=(tap == 8))
                    s["p2"] = p2
                # ---- stage C (job i-2): m2relu, conv3, residual, store ----
                if 0 <= i - 2 < len(jobs):
                    s = state.pop(i - 2)
                    b, pair, py = s["b"], s["pair"], s["py"]
                    xviews = s["xviews"]
                    mseg = mkt[:, (b * 2 + pair) * 49 + py * 7:
                               (b * 2 + pair) * 49 + py * 7 + 7]
                    mbc = mseg.unsqueeze(2).broadcast_to([128, 7, 64])
                    m2 = m2pool.tile([128, 448], bf16)
                    m2v = m2[:].rearrange("p (a b) -> p a b", a=7)
                    p2v3 = s["p2"][:].rearrange("p (a b) -> p a b", a=7)
                    nc.vector.scalar_tensor_tensor(m2v, p2v3, 0.0, mbc, Max, Mult)
                    m2r = m2[:]
                    ot = opool.tile([128, 4 * 448], bf16)
                    for mt in range(2):
                        for gi in range(2):
                            blk = (pair * 2 + gi) * 2 + mt
                            ct = 2 * gi + mt
                            p3 = ps3.tile([128, 448], f32)
                            nc.tensor.matmul(
                                p3[:],
                                w3t[64 * gi:64 * gi + 64, blk * 128:(blk + 1) * 128],
                                m2r[64 * gi:64 * gi + 64, :])
                            t3 = t3pool.tile([128, 448], bf16)
                            t3v = t3[:].rearrange("p (a b c) -> p a b c", b=8, c=8)
                            p3v = p3[:].rearrange("p (a b c) -> p a b c", b=8, c=8)
                            nc.vector.tensor_add(t3v, p3v, xviews[ct])
                            otv = ot[:, ct * 448:(ct + 1) * 448].rearrange(
                                "p (y px x) -> p px y x", y=8, px=7, x=8)
                            nc.scalar.activation(otv, t3v, Relu)
                    store_eng.dma_start(
                        oap[b, 512 * pair:512 * pair + 512,
                            8 * py:8 * py + 8, :]
                        .rearrange("(u c) h w -> c u (h w)", u=4),
                        ot[:].rearrange("p (u s) -> p u s", u=4))
                    if py == MS - 1:
                        del xts_for_set[s["si"]]
    nc.compile()
    return nc


def _get_program():
    if "nc" not in _CACHE:
        _CACHE["nc"] = _build_program()
    return _CACHE["nc"]


def make_in_maps(x, mask, w1, w2, w3):
    bf16 = _bf16()
    x = np.ascontiguousarray(np.asarray(x, np.float32)).astype(bf16)
    mask = np.asarray(mask, np.float32)
    w1s, w2s, w3s = _pack_weights(np.asarray(w1, np.float32),
                                  np.asarray(w2, np.float32),
                                  np.asarray(w3, np.float32))
    w1s, w2s, w3s = w1s.astype(bf16), w2s.astype(bf16), w3s.astype(bf16)
    in_maps = []
    for k in range(NCORES):
        in_maps.append({
            "x": x[BPC * k:BPC * (k + 1)],
            "maskrep": _pack_mask(mask[BPC * k:BPC * (k + 1)]),
            "w1s": w1s, "w2s": w2s, "w3s": w3s,
        })
    return in_maps


def kernel(x, mask, w1, w2, w3):
    from concourse import bass_utils

    in_maps = make_in_maps(x, mask, w1, w2, w3)
    nc = _get_program()
    res = bass_utils.run_bass_kernel_spmd(nc, in_maps, core_ids=list(range(NCORES)))
    out = np.concatenate([res.results[k]["out"] for k in range(NCORES)], axis=0)
    return out.astype(np.float32)


# revision 13
# speedup vs baseline: 4.0264x; 1.0918x over previous
"""Trainium2 Bass kernel for masked grouped-bottleneck (moe_routing patch refine).

Full computation:
  x [16,1024,56,56] is split into a 7x7 grid of 8x8 patches; per patch a
  grouped (G=4) bottleneck conv1(1x1)->relu->conv2(3x3, per-patch pad)->relu
  ->conv3(1x1) runs; the result is zeroed for non-selected (b, group, patch)
  combos per `mask`, un-patchified, added to x (residual) and relu'd.

Sharding: data-parallel over batch, 2 images per core across 8 cores.

All tensors ride in bf16 (inputs downcast on the host, output upcast back to
fp32 on the host): the 2e-2 rel-err budget dwarfs bf16's ~2e-3, and halving
the HBM bytes moves the DMA floor from ~143us to ~71us per core while the PE
stays at 1 cycle/row either way (fp32r is also 1 cycle/row at N=448).

Weights are repacked on the host into PE-friendly lhsT layouts (block-diagonal
over group pairs so conv2 runs dense K=128/M=128 matmuls). The routing mask is
applied right after conv2: every conv is patch-local and bias-free, so zeroing
m2 for a (group, patch) is exactly equivalent to zeroing the conv3 output.

Pipeline per (batch, group-pair) macro-iteration, streaming 7 patch rows:
  conv1 (PE, 4 accumulating blockdiag matmuls) -> relu (ACT) into the zero-
  padded m1 interior (borders memset once, first 3 jobs only — they are never
  overwritten) -> conv2 as 9 accumulating taps over shifted padded views
  (walrus requires matmul OUT APs to canonicalize to <=3 dims, so clipped
  PSUM sub-views are not an option) -> fused mask*relu (one DVE
  scalar_tensor_tensor m2 = (p2 max 0) * mask) -> conv3 (PE) -> residual add
  (DVE) -> relu into a 4-slab output tile, blocks 0/1 on ACT and blocks 2/3 on
  Pool (gpsimd), keeping both under the PE roofline -> one store per patch row.

Engine budget per core (28 jobs, cost-model): PE 17x448 rows/job = 89us,
DVE (stt + 4 residual adds, PSUM-bound so no 2x mode) = 83us, ACT = 46us,
Pool = 40us, DMA 25.7MB = 71us.

DMA: x loads (one 3.2MB dma_start per set) and the batched stores ride the SP
(sync) HWDGE ring; loads for a set are issued two sets ahead (xpool bufs=3).
Stores must NOT go on the ACT ring: a store's sem wait would hold the ACT
sequencer and block the next row's relu decodes.
"""
import numpy as np

_CACHE = {}

B, C, H, W = 16, 1024, 56, 56
G, MS, HP = 4, 7, 8
MID = 256
NCORES = 8
BPC = B // NCORES   # batches per core


def _bf16():
    from concourse import mybir
    return mybir.dt.np(mybir.dt.bfloat16)


def _pack_weights(w1, w2, w3):
    w1s = np.zeros((128, 2 * 4 * 128), np.float32)
    for pair in range(2):
        for j in range(4):            # K-tile over the pair's 512 input chans
            gi, kt = j // 2, j % 2
            g = 2 * pair + gi
            Wg = w1[64 * g:64 * g + 64, 128 * kt:128 * kt + 128, 0, 0]
            w1s[:, (pair * 4 + j) * 128 + 64 * gi:(pair * 4 + j) * 128 + 64 * gi + 64] = Wg.T
    w2s = np.zeros((128, 2 * 9 * 128), np.float32)
    for pair in range(2):
        for tap in range(9):
            dy, dx = tap // 3, tap % 3
            for gi in range(2):
                g = 2 * pair + gi
                Wg = w2[64 * g:64 * g + 64, :, dy, dx]
                w2s[64 * gi:64 * gi + 64,
                    (pair * 9 + tap) * 128 + 64 * gi:(pair * 9 + tap) * 128 + 64 * gi + 64] = Wg.T
    w3s = np.zeros((128, 8 * 128), np.float32)
    for pair in range(2):
        for gi in range(2):
            g = 2 * pair + gi
            for mt in range(2):
                Wg = w3[256 * g + 128 * mt:256 * g + 128 * (mt + 1), :, 0, 0]
                blk = (pair * 2 + gi) * 2 + mt
                w3s[64 * gi:64 * gi + 64, blk * 128:(blk + 1) * 128] = Wg.T
    return w1s, w2s, w3s


def _pack_mask(mask_b):
    # mask_b: [BPC, 4, 7, 7] -> [128, BPC*2*49], row r belongs to group 2*pair + r//64
    m = np.zeros((128, BPC * 2 * 49), np.float32)
    mb = (mask_b > 0).astype(np.float32).reshape(BPC, 4, 49)
    for b in range(BPC):
        for pair in range(2):
            seg = slice((b * 2 + pair) * 49, (b * 2 + pair + 1) * 49)
            m[0:64, seg] = mb[b, 2 * pair]
            m[64:128, seg] = mb[b, 2 * pair + 1]
    return m


def _build_program(reps=1, store_engine="pool"):
    import concourse.bacc as bacc
    import concourse.mybir as mybir
    import concourse.tile as tile

    f32 = mybir.dt.float32
    bf16 = mybir.dt.bfloat16
    Relu = mybir.ActivationFunctionType.Relu
    Max = mybir.AluOpType.max
    Mult = mybir.AluOpType.mult

    nc = bacc.Bacc("TRN2", target_bir_lowering=False, debug=False)
    x_d = nc.dram_tensor("x", [BPC, C, H, W], bf16, kind="ExternalInput")
    mk_d = nc.dram_tensor("maskrep", [128, BPC * 2 * 49], f32, kind="ExternalInput")
    w1_d = nc.dram_tensor("w1s", [128, 1024], bf16, kind="ExternalInput")
    w2_d = nc.dram_tensor("w2s", [128, 2304], bf16, kind="ExternalInput")
    w3_d = nc.dram_tensor("w3s", [128, 1024], bf16, kind="ExternalInput")
    out_d = nc.dram_tensor("out", [BPC, C, H, W], bf16, kind="ExternalOutput")

    xap = x_d.ap()
    oap = out_d.ap()

    with tile.TileContext(nc) as tc:
        with (
            tc.tile_pool(name="wpool", bufs=1) as wpool,
            tc.tile_pool(name="xpool", bufs=4) as xpool,
            tc.tile_pool(name="m1pool", bufs=3) as m1pool,
            tc.tile_pool(name="m2pool", bufs=3) as m2pool,
            tc.tile_pool(name="t3pool", bufs=3) as t3pool,
            tc.tile_pool(name="opool", bufs=3) as opool,
            tc.tile_pool(name="ps1", bufs=2, space="PSUM") as ps1,
            tc.tile_pool(name="ps2", bufs=2, space="PSUM") as ps2,
            tc.tile_pool(name="ps3", bufs=4, space="PSUM") as ps3,
        ):
            w1t = wpool.tile([128, 1024], bf16, tag="w1")
            w2t = wpool.tile([128, 2304], bf16, tag="w2")
            w3t = wpool.tile([128, 1024], bf16, tag="w3")
            mkt = wpool.tile([128, BPC * 2 * 49], f32, tag="mk")
            # weights go on the ACT HWDGE ring so they don't queue ahead
            # of the first x-tile loads on the sync ring at startup
            nc.scalar.dma_start(w1t[:], w1_d.ap())
            nc.scalar.dma_start(w2t[:], w2_d.ap())
            nc.scalar.dma_start(w3t[:], w3_d.ap())
            nc.scalar.dma_start(mkt[:], mk_d.ap())

            store_eng = {"scalar": nc.scalar, "pool": nc.gpsimd,
                         "sync": nc.sync}[store_engine]

            # (b, pair) macro-sets, 7 patch-row jobs each, pipelined 2 deep:
            # iteration i runs conv1[i], relu1+conv2[i-1], and
            # stt+conv3+residual+store[i-2] so no engine queue ever waits
            # on a same-row cross-engine dependency.
            set_list = [(b, pair)
                        for _ in range(reps)
                        for b in range(BPC)
                        for pair in range(2)]
            xts_for_set = {}

            def load_set(si):
                b, pair = set_list[si]
                xt = xpool.tile([128, 4 * H * W], bf16, tag="xt", name=f"xt{si}")
                nc.sync.dma_start(
                    xt[:].rearrange("c (u s) -> c u s", u=4),
                    xap[b, 512 * pair:512 * pair + 512]
                    .rearrange("(u c) h w -> c u (h w)", u=4))
                xts_for_set[si] = xt

            jobs = [(si, py) for si in range(len(set_list)) for py in range(MS)]
            for si0 in range(min(3, len(set_list))):
                load_set(si0)
            state = {}
            cur_ot = [None]

            for i in range(len(jobs) + 2):
                # ---- stage A (job i): conv1 ----
                if i < len(jobs):
                    si, py = jobs[i]
                    if py == 0 and si + 3 < len(set_list):
                        load_set(si + 3)
                    b, pair = set_list[si]
                    xt = xts_for_set[si]
                    xviews = [
                        xt[:].rearrange("p (u py y px x) -> p u py px y x",
                                        u=4, py=7, y=8, px=7, x=8)[:, u, py]
                        for u in range(4)
                    ]
                    p1 = ps1.tile([128, 448], f32)
                    for j in range(4):
                        nc.tensor.matmul(
                            p1[:],
                            w1t[:, (pair * 4 + j) * 128:(pair * 4 + j + 1) * 128],
                            xviews[j],
                            start=(j == 0), stop=(j == 3))
                    state[i] = dict(si=si, py=py, b=b, pair=pair,
                                    xviews=xviews, p1=p1)
                # ---- stage B (job i-1): relu into padded m1, conv2 ----
                if 0 <= i - 1 < len(jobs):
                    s = state[i - 1]
                    b, pair, py = s["b"], s["pair"], s["py"]
                    m1 = m1pool.tile([128, 700], bf16)
                    if i - 1 < 3:
                        # zero the patch borders once per physical buffer;
                        # only the 8x8 interior is ever rewritten after this
                        nc.gpsimd.memset(m1[:].bitcast(f32), 0.0)
                    m1v = m1[:].rearrange("p (a b c) -> p a b c", b=10, c=10)
                    p1v = s["p1"][:].rearrange("p (a b c) -> p a b c", b=8, c=8)
                    nc.scalar.activation(m1v[:, :, 1:9, 1:9], p1v, Relu)
                    p2 = ps2.tile([128, 448], f32)
                    for tap in range(9):
                        dy, dx = tap // 3, tap % 3
                        nc.tensor.matmul(
                            p2[:],
                            w2t[:, (pair * 9 + tap) * 128:(pair * 9 + tap + 1) * 128],
                            m1v[:, :, dy:dy + 8, dx:dx + 8],
                            start=(tap == 0), stop=(tap == 8))
                    s["p2"] = p2
                # ---- stage C (job i-2): m2relu, conv3, residual, store ----
                if 0 <= i - 2 < len(jobs):
                    s = state.pop(i - 2)
                    b, pair, py = s["b"], s["pair"], s["py"]
                    xviews = s["xviews"]
                    mseg = mkt[:, (b * 2 + pair) * 49 + py * 7:
                               (b * 2 + pair) * 49 + py * 7 + 7]
                    mbc = mseg.unsqueeze(2).broadcast_to([128, 7, 64])
                    m2 = m2pool.tile([128, 448], bf16)
                    m2v = m2[:].rearrange("p (a b) -> p a b", a=7)
                    p2v3 = s["p2"][:].rearrange("p (a b) -> p a b", a=7)
                    nc.vector.scalar_tensor_tensor(m2v, p2v3, 0.0, mbc, Max, Mult)
                    m2r = m2[:]
                    # ot holds up to TWO patch rows (r = py % 2); one store
                    # per row pair halves the SWDGE count and doubles the
                    # contiguous HBM run to 1792B. Row 6 stores alone.
                    r = py % 2
                    if r == 0:
                        cur_ot[0] = opool.tile([128, 4 * 2 * 448], bf16,
                                               name=f"ot{i}", tag="ot")
                    ot = cur_ot[0]
                    for mt in range(2):
                        for gi in range(2):
                            blk = (pair * 2 + gi) * 2 + mt
                            ct = 2 * gi + mt
                            p3 = ps3.tile([128, 448], f32)
                            nc.tensor.matmul(
                                p3[:],
                                w3t[64 * gi:64 * gi + 64, blk * 128:(blk + 1) * 128],
                                m2r[64 * gi:64 * gi + 64, :])
                            t3 = t3pool.tile([128, 448], bf16)
                            t3v = t3[:].rearrange("p (a b c) -> p a b c", b=8, c=8)
                            p3v = p3[:].rearrange("p (a b c) -> p a b c", b=8, c=8)
                            nc.vector.tensor_add(t3v, p3v, xviews[ct])
                            otv = ot[:, (ct * 2 + r) * 448:
                                      (ct * 2 + r + 1) * 448].rearrange(
                                "p (y px x) -> p px y x", y=8, px=7, x=8)
                            nc.scalar.activation(otv, t3v, Relu)
                    if r == 1 or py == MS - 1:
                        nrows = r + 1
                        store_eng.dma_start(
                            oap[b, 512 * pair:512 * pair + 512,
                                8 * (py - r):8 * (py + 1), :]
                            .rearrange("(u c) h w -> c u (h w)", u=4),
                            ot[:].rearrange("p (u r2 s) -> p u r2 s",
                                            u=4, r2=2)[:, :, 0:nrows]
                            .rearrange("p u r2 s -> p u (r2 s)"))
                    if py == MS - 1:
                        del xts_for_set[s["si"]]
    nc.compile()
    return nc


def _get_program():
    if "nc" not in _CACHE:
        _CACHE["nc"] = _build_program()
    return _CACHE["nc"]


def make_in_maps(x, mask, w1, w2, w3):
    bf16 = _bf16()
    x = np.ascontiguousarray(np.asarray(x, np.float32)).astype(bf16)
    mask = np.asarray(mask, np.float32)
    w1s, w2s, w3s = _pack_weights(np.asarray(w1, np.float32),
                                  np.asarray(w2, np.float32),
                                  np.asarray(w3, np.float32))
    w1s, w2s, w3s = w1s.astype(bf16), w2s.astype(bf16), w3s.astype(bf16)
    in_maps = []
    for k in range(NCORES):
        in_maps.append({
            "x": x[BPC * k:BPC * (k + 1)],
            "maskrep": _pack_mask(mask[BPC * k:BPC * (k + 1)]),
            "w1s": w1s, "w2s": w2s, "w3s": w3s,
        })
    return in_maps


def kernel(x, mask, w1, w2, w3):
    from concourse import bass_utils

    in_maps = make_in_maps(x, mask, w1, w2, w3)
    nc = _get_program()
    res = bass_utils.run_bass_kernel_spmd(nc, in_maps, core_ids=list(range(NCORES)))
    out = np.concatenate([res.results[k]["out"] for k in range(NCORES)], axis=0)
    return out.astype(np.float32)


# revision 16
# speedup vs baseline: 4.0488x; 1.0056x over previous
"""Trainium2 Bass kernel for masked grouped-bottleneck (moe_routing patch refine).

Full computation:
  x [16,1024,56,56] is split into a 7x7 grid of 8x8 patches; per patch a
  grouped (G=4) bottleneck conv1(1x1)->relu->conv2(3x3, per-patch pad)->relu
  ->conv3(1x1) runs; the result is zeroed for non-selected (b, group, patch)
  combos per `mask`, un-patchified, added to x (residual) and relu'd.

Sharding: data-parallel over batch, 2 images per core across 8 cores.

All tensors ride in bf16 (inputs downcast on the host, output upcast back to
fp32 on the host): the 2e-2 rel-err budget dwarfs bf16's ~2e-3, and halving
the HBM bytes moves the DMA floor from ~143us to ~71us per core while the PE
stays at 1 cycle/row either way (fp32r is also 1 cycle/row at N=448).

Weights are repacked on the host into PE-friendly lhsT layouts (block-diagonal
over group pairs so conv2 runs dense K=128/M=128 matmuls). The routing mask is
applied right after conv2: every conv is patch-local and bias-free, so zeroing
m2 for a (group, patch) is exactly equivalent to zeroing the conv3 output.

Pipeline per (batch, group-pair) macro-iteration, streaming 7 patch rows:
  conv1 (PE, 4 accumulating blockdiag matmuls) -> relu (ACT) into the zero-
  padded m1 interior (borders memset once, first 3 jobs only — they are never
  overwritten) -> conv2 as 9 accumulating taps over shifted padded views
  (walrus requires matmul OUT APs to canonicalize to <=3 dims, so clipped
  PSUM sub-views are not an option) -> fused mask*relu (one DVE
  scalar_tensor_tensor m2 = (p2 max 0) * mask) -> conv3 (PE) -> residual add
  (DVE) -> relu into a 4-slab output tile, blocks 0/1 on ACT and blocks 2/3 on
  Pool (gpsimd), keeping both under the PE roofline -> one store per patch row.

Engine budget per core (28 jobs, cost-model): PE 17x448 rows/job = 89us,
DVE (stt + 4 residual adds, PSUM-bound so no 2x mode) = 83us, ACT = 46us,
Pool = 40us, DMA 25.7MB = 71us.

DMA: x loads (one 3.2MB dma_start per set) and the batched stores ride the SP
(sync) HWDGE ring; loads for a set are issued two sets ahead (xpool bufs=3).
Stores must NOT go on the ACT ring: a store's sem wait would hold the ACT
sequencer and block the next row's relu decodes.
"""
import numpy as np

_CACHE = {}

B, C, H, W = 16, 1024, 56, 56
G, MS, HP = 4, 7, 8
MID = 256
NCORES = 8
BPC = B // NCORES   # batches per core


def _bf16():
    from concourse import mybir
    return mybir.dt.np(mybir.dt.bfloat16)


def _pack_weights(w1, w2, w3):
    w1s = np.zeros((128, 2 * 4 * 128), np.float32)
    for pair in range(2):
        for j in range(4):            # K-tile over the pair's 512 input chans
            gi, kt = j // 2, j % 2
            g = 2 * pair + gi
            Wg = w1[64 * g:64 * g + 64, 128 * kt:128 * kt + 128, 0, 0]
            w1s[:, (pair * 4 + j) * 128 + 64 * gi:(pair * 4 + j) * 128 + 64 * gi + 64] = Wg.T
    w2s = np.zeros((128, 2 * 9 * 128), np.float32)
    for pair in range(2):
        for tap in range(9):
            dy, dx = tap // 3, tap % 3
            for gi in range(2):
                g = 2 * pair + gi
                Wg = w2[64 * g:64 * g + 64, :, dy, dx]
                w2s[64 * gi:64 * gi + 64,
                    (pair * 9 + tap) * 128 + 64 * gi:(pair * 9 + tap) * 128 + 64 * gi + 64] = Wg.T
    w3s = np.zeros((128, 8 * 128), np.float32)
    for pair in range(2):
        for gi in range(2):
            g = 2 * pair + gi
            for mt in range(2):
                Wg = w3[256 * g + 128 * mt:256 * g + 128 * (mt + 1), :, 0, 0]
                blk = (pair * 2 + gi) * 2 + mt
                w3s[64 * gi:64 * gi + 64, blk * 128:(blk + 1) * 128] = Wg.T
    return w1s, w2s, w3s


def _pack_mask(mask_b):
    # mask_b: [BPC, 4, 7, 7] -> [128, BPC*2*49], row r belongs to group 2*pair + r//64
    m = np.zeros((128, BPC * 2 * 49), np.float32)
    mb = (mask_b > 0).astype(np.float32).reshape(BPC, 4, 49)
    for b in range(BPC):
        for pair in range(2):
            seg = slice((b * 2 + pair) * 49, (b * 2 + pair + 1) * 49)
            m[0:64, seg] = mb[b, 2 * pair]
            m[64:128, seg] = mb[b, 2 * pair + 1]
    return m


def _build_program(reps=1, store_engine="pool"):
    import concourse.bacc as bacc
    import concourse.mybir as mybir
    import concourse.tile as tile

    f32 = mybir.dt.float32
    bf16 = mybir.dt.bfloat16
    Relu = mybir.ActivationFunctionType.Relu
    Max = mybir.AluOpType.max
    Mult = mybir.AluOpType.mult

    nc = bacc.Bacc("TRN2", target_bir_lowering=False, debug=False)
    x_d = nc.dram_tensor("x", [BPC, C, H, W], bf16, kind="ExternalInput")
    mk_d = nc.dram_tensor("maskrep", [128, BPC * 2 * 49], f32, kind="ExternalInput")
    w1_d = nc.dram_tensor("w1s", [128, 1024], bf16, kind="ExternalInput")
    w2_d = nc.dram_tensor("w2s", [128, 2304], bf16, kind="ExternalInput")
    w3_d = nc.dram_tensor("w3s", [128, 1024], bf16, kind="ExternalInput")
    out_d = nc.dram_tensor("out", [BPC, C, H, W], bf16, kind="ExternalOutput")

    xap = x_d.ap()
    oap = out_d.ap()

    with tile.TileContext(nc) as tc:
        with (
            tc.tile_pool(name="wpool", bufs=1) as wpool,
            tc.tile_pool(name="xpool", bufs=3) as xpool,
            tc.tile_pool(name="m1pool", bufs=3) as m1pool,
            tc.tile_pool(name="m2pool", bufs=3) as m2pool,
            tc.tile_pool(name="t3pool", bufs=3) as t3pool,
            tc.tile_pool(name="opool", bufs=3) as opool,
            tc.tile_pool(name="ps1", bufs=2, space="PSUM") as ps1,
            tc.tile_pool(name="ps2", bufs=2, space="PSUM") as ps2,
            tc.tile_pool(name="ps3", bufs=4, space="PSUM") as ps3,
        ):
            w1t = wpool.tile([128, 1024], bf16, tag="w1")
            w2t = wpool.tile([128, 2304], bf16, tag="w2")
            w3t = wpool.tile([128, 1024], bf16, tag="w3")
            mkt = wpool.tile([128, BPC * 2 * 49], f32, tag="mk")
            # weights go on the ACT HWDGE ring so they don't queue ahead
            # of the first x-tile loads on the sync ring at startup
            nc.scalar.dma_start(w1t[:], w1_d.ap())
            nc.scalar.dma_start(w2t[:], w2_d.ap())
            nc.scalar.dma_start(w3t[:], w3_d.ap())
            nc.scalar.dma_start(mkt[:], mk_d.ap())

            store_eng = {"scalar": nc.scalar, "pool": nc.gpsimd,
                         "sync": nc.sync}[store_engine]

            # (b, pair) macro-sets, 7 patch-row jobs each, pipelined 2 deep:
            # iteration i runs conv1[i], relu1+conv2[i-1], and
            # stt+conv3+residual+store[i-2] so no engine queue ever waits
            # on a same-row cross-engine dependency.
            set_list = [(b, pair)
                        for _ in range(reps)
                        for b in range(BPC)
                        for pair in range(2)]
            xts_for_set = {}

            def load_set(si):
                b, pair = set_list[si]
                xt = xpool.tile([128, 4 * H * W], bf16, tag="xt", name=f"xt{si}")
                nc.sync.dma_start(
                    xt[:].rearrange("c (u s) -> c u s", u=4),
                    xap[b, 512 * pair:512 * pair + 512]
                    .rearrange("(u c) h w -> c u (h w)", u=4))
                xts_for_set[si] = xt

            jobs = [(si, py) for si in range(len(set_list)) for py in range(MS)]
            load_set(0)
            if len(set_list) > 1:
                load_set(1)
            state = {}
            cur_ot = [None]
            cur_t3 = [None]

            for i in range(len(jobs) + 2):
                # ---- stage A (job i): conv1 ----
                if i < len(jobs):
                    si, py = jobs[i]
                    if py == 0 and si + 2 < len(set_list):
                        load_set(si + 2)
                    b, pair = set_list[si]
                    xt = xts_for_set[si]
                    xviews = [
                        xt[:].rearrange("p (u py y px x) -> p u py px y x",
                                        u=4, py=7, y=8, px=7, x=8)[:, u, py]
                        for u in range(4)
                    ]
                    p1 = ps1.tile([128, 448], f32)
                    for j in range(4):
                        nc.tensor.matmul(
                            p1[:],
                            w1t[:, (pair * 4 + j) * 128:(pair * 4 + j + 1) * 128],
                            xviews[j],
                            start=(j == 0), stop=(j == 3))
                    state[i] = dict(si=si, py=py, b=b, pair=pair,
                                    xviews=xviews, p1=p1)
                # ---- stage B (job i-1): relu into padded m1, conv2 ----
                if 0 <= i - 1 < len(jobs):
                    s = state[i - 1]
                    b, pair, py = s["b"], s["pair"], s["py"]
                    m1 = m1pool.tile([128, 700], bf16)
                    if i - 1 < 3:
                        # zero the patch borders once per physical buffer;
                        # only the 8x8 interior is ever rewritten after this
                        nc.gpsimd.memset(m1[:].bitcast(f32), 0.0)
                    m1v = m1[:].rearrange("p (a b c) -> p a b c", b=10, c=10)
                    p1v = s["p1"][:].rearrange("p (a b c) -> p a b c", b=8, c=8)
                    nc.scalar.activation(m1v[:, :, 1:9, 1:9], p1v, Relu)
                    p2 = ps2.tile([128, 448], f32)
                    for tap in range(9):
                        dy, dx = tap // 3, tap % 3
                        nc.tensor.matmul(
                            p2[:],
                            w2t[:, (pair * 9 + tap) * 128:(pair * 9 + tap + 1) * 128],
                            m1v[:, :, dy:dy + 8, dx:dx + 8],
                            start=(tap == 0), stop=(tap == 8))
                    s["p2"] = p2
                # ---- stage C (job i-2): m2relu, conv3, residual, store ----
                if 0 <= i - 2 < len(jobs):
                    s = state.pop(i - 2)
                    b, pair, py = s["b"], s["pair"], s["py"]
                    xviews = s["xviews"]
                    mseg = mkt[:, (b * 2 + pair) * 49 + py * 7:
                               (b * 2 + pair) * 49 + py * 7 + 7]
                    mbc = mseg.unsqueeze(2).broadcast_to([128, 7, 64])
                    m2 = m2pool.tile([128, 448], bf16)
                    m2v = m2[:].rearrange("p (a b) -> p a b", a=7)
                    p2v3 = s["p2"][:].rearrange("p (a b) -> p a b", a=7)
                    nc.vector.scalar_tensor_tensor(m2v, p2v3, 0.0, mbc, Max, Mult)
                    m2r = m2[:]
                    # ot holds up to TWO patch rows (r = py % 2); one store
                    # per row pair halves the SWDGE count and doubles the
                    # contiguous HBM run to 1792B. Row 6 stores alone.
                    r = py % 2
                    if r == 0:
                        cur_ot[0] = opool.tile([128, 4 * 2 * 448], bf16,
                                               name=f"ot{i}", tag="ot")
                        cur_t3[0] = t3pool.tile([128, 4 * 2 * 448], bf16,
                                                name=f"t3{i}", tag="t3")
                    ot = cur_ot[0]
                    t3 = cur_t3[0]
                    for mt in range(2):
                        for gi in range(2):
                            blk = (pair * 2 + gi) * 2 + mt
                            ct = 2 * gi + mt
                            p3 = ps3.tile([128, 448], f32)
                            nc.tensor.matmul(
                                p3[:],
                                w3t[64 * gi:64 * gi + 64, blk * 128:(blk + 1) * 128],
                                m2r[64 * gi:64 * gi + 64, :])
                            # t3 shares ot's (u, r, y, px, x) layout so one
                            # contiguous-view ACT relu per job drains all 4
                            # blocks; the DVE add writes it via a strided
                            # (px,y,x)->(y,px,x) out view.
                            seg = (ct * 2 + r) * 448
                            t3v = t3[:, seg:seg + 448].rearrange(
                                "p (y px x) -> p px y x", y=8, px=7, x=8)
                            p3v = p3[:].rearrange("p (a b c) -> p a b c", b=8, c=8)
                            nc.vector.tensor_add(t3v, p3v, xviews[ct])
                    t3j = t3[:].rearrange("p (u r2 s) -> p u r2 s",
                                          u=4, r2=2)[:, :, r]
                    otj = ot[:].rearrange("p (u r2 s) -> p u r2 s",
                                          u=4, r2=2)[:, :, r]
                    nc.scalar.activation(otj, t3j, Relu)
                    if r == 1 or py == MS - 1:
                        nrows = r + 1
                        store_eng.dma_start(
                            oap[b, 512 * pair:512 * pair + 512,
                                8 * (py - r):8 * (py + 1), :]
                            .rearrange("(u c) h w -> c u (h w)", u=4),
                            ot[:].rearrange("p (u r2 s) -> p u r2 s",
                                            u=4, r2=2)[:, :, 0:nrows]
                            .rearrange("p u r2 s -> p u (r2 s)"))
                    if py == MS - 1:
                        del xts_for_set[s["si"]]
    nc.compile()
    return nc


def _get_program():
    if "nc" not in _CACHE:
        _CACHE["nc"] = _build_program()
    return _CACHE["nc"]


def make_in_maps(x, mask, w1, w2, w3):
    bf16 = _bf16()
    x = np.ascontiguousarray(np.asarray(x, np.float32)).astype(bf16)
    mask = np.asarray(mask, np.float32)
    w1s, w2s, w3s = _pack_weights(np.asarray(w1, np.float32),
                                  np.asarray(w2, np.float32),
                                  np.asarray(w3, np.float32))
    w1s, w2s, w3s = w1s.astype(bf16), w2s.astype(bf16), w3s.astype(bf16)
    in_maps = []
    for k in range(NCORES):
        in_maps.append({
            "x": x[BPC * k:BPC * (k + 1)],
            "maskrep": _pack_mask(mask[BPC * k:BPC * (k + 1)]),
            "w1s": w1s, "w2s": w2s, "w3s": w3s,
        })
    return in_maps


def kernel(x, mask, w1, w2, w3):
    from concourse import bass_utils

    in_maps = make_in_maps(x, mask, w1, w2, w3)
    nc = _get_program()
    res = bass_utils.run_bass_kernel_spmd(nc, in_maps, core_ids=list(range(NCORES)))
    out = np.concatenate([res.results[k]["out"] for k in range(NCORES)], axis=0)
    return out.astype(np.float32)
